# revision 1
# baseline (speedup 1.0000x reference)
"""Trainium2 Bass kernel for 0.7*BCEWithLogits + 0.3*MultiLabelMarginLoss.

Math (per row of N = B*T rows, V = 128 classes; output = mean over rows):
  bce_row = mean_n[ softplus(x_n) - x_n*t_n ]
            softplus(x) = relu(x) + log1p(exp(-|x|));  sum relu = (sum x + sum |x|)/2
  mlm_row = (1/V) sum_{p in pos} sum_{n in neg} relu(1 - x_p + x_n)

Only global sums matter (scalar output), so reductions accumulate into
per-block/per-group columns or PSUM and combine once per core.

Positive logits (<= ~11 per row here) are extracted per 128-row block with
vector.max (top-8, sorted) + match_replace + vector.max into a raw table
t' = x_pos + 512 (pads = 0). The V^2 pairwise hinge collapses to S slots
per row, one fused custom DVE instruction per block:
    z[p,k,n] = select(t'[p,k] > 256, relu(u[p,n] - t'[p,k] + 513), 0)
    accum_out[p] = sum z        (u = x with positives pushed to -512)
A second custom op folds sum(x*t) per 4-block group into one instruction.

Everything else is batched per 4-block group to amortize fixed costs:
one 512 KiB DMA; custom fused DVE ops for the two maskings
pxB = (x+512)*pos and u = x*(1-pos) - 512*pos over [128, 512] strided views
(group 0 runs per-block for pipeline fill); one Abs/Exp/Ln chain on ACT over
[128, 512] with group accum_out (single pinned activation-table set, loaded
once); one PE column-sum matmul stream for sum(x). GPSIMD is kept idle on
purpose: concurrent GpSimd SBUF traffic slows DVE ops ~2x (port sharing).

Sharding: host sorts rows by positive count, deals them round-robin to the
8 cores (identical npos profile per core), interleaves x|targets, and lays
the core's 16 blocks side-by-side as a [128, 16*256] array so each group is
one contiguous DMA. Block b needs S_b hinge slots; the schedule derives from
the npos histogram, one cached NEFF per distinct schedule. All arithmetic is
on device; the host only permutes/shards and sums the 8 core partials.
"""

import sys

sys.path.insert(0, "/opt/trn_rl_repo")

import numpy as np

import concourse.bacc as bacc
import concourse.tile as tile
from concourse import mybir
from concourse.bass_utils import run_bass_kernel_spmd

F32 = mybir.dt.float32
ALU = mybir.AluOpType
ACTF = mybir.ActivationFunctionType
AXL = mybir.AxisListType

B, T, V = 16, 1024, 128
ROWS = B * T
N_CORES = 8
RPC = ROWS // N_CORES             # 2048 rows per core
P = 128                           # rows per block
NBLK = RPC // P                   # 16 blocks
GRP = 4                           # blocks per group
NGRP = NBLK // GRP
CB = 2 * V                        # columns per block in the packed layout
CG = GRP * CB                     # columns per group

BIG = 512.0
BCE_W = 0.7
MLM_W = 0.3


def _register_ops():
    from concourse import dve_ops as dops
    from concourse.dve_spec import (
        Spec, Src0, Src1, AluOp, relu, select, Zero, One, C0, C1,
    )

    if hasattr(dops, "ANT_KERNEL_OPS"):
        return dops.ANT_KERNEL_OPS

    def _zref(in0, in1, c0, c1, c2):
        i0 = in0.astype(np.float32).reshape(in0.shape[0], -1)
        t = in1.astype(np.float32).reshape(in0.shape[0], -1)
        b = np.where(t > c0, np.maximum(i0 - t + c1, 0.0), 0.0)
        return b, b.sum(-1, keepdims=True)

    z_spec = Spec(
        body=select(Src1 > C0, relu(Src0 - Src1 + C1), Zero),
        accum=AluOp.ADD, reference=_zref,
    )

    def _xtref(in0, in1, c0, c1, c2):
        t = in0.astype(np.float32).reshape(in0.shape[0], -1)
        b = np.where(t > c0, t - c1, 0.0)
        return b, b.sum(-1, keepdims=True)

    xt_spec = Spec(
        body=select(Src0 > C0, Src0 - C1, Zero),
        accum=AluOp.ADD, reference=_xtref,
    )

    def _pxref(in0, in1, c0, c1, c2):
        i0 = in0.astype(np.float32).reshape(in0.shape[0], -1)
        i1 = in1.astype(np.float32).reshape(in0.shape[0], -1)
        return (i0 + c0) * i1

    px_spec = Spec(body=(Src0 + C0) * Src1, reference=_pxref)

    def _uoref(in0, in1, c0, c1, c2):
        i0 = in0.astype(np.float32).reshape(in0.shape[0], -1)
        i1 = in1.astype(np.float32).reshape(in0.shape[0], -1)
        return i0 * (1.0 - i1) - c0 * i1

    uo_spec = Spec(body=Src0 * (One - Src1) - C0 * Src1, reference=_uoref)

    ops = {}
    for name, spec in (
        ("Z_HINGE2_ANT", z_spec),
        ("XT_SUM_ANT", xt_spec),
        ("PX_MASK_ANT", px_spec),
        ("U_MASK_ANT", uo_spec),
    ):
        opc = max(dops._SUB_OPCODE_FOR_NAME.values()) + 1
        shas = {}
        for ver in ("v3", "v4"):
            r = dops.DveOpSpec(
                name=name, opcode=opc,
                uops=dops.lower(spec, ver=ver), rd1_en=dops.has_src1(spec),
            )
            shas[ver] = r.sha(ver)
        op = dops.DveOp(name, spec, subdim=False, uops_sha=shas)
        dops.OPS.append(op)
        dops.CUSTOM_DVE_SPECS[name] = spec
        dops._SUB_OPCODE_FOR_NAME[name] = opc
        ops[name] = op
    dops.ANT_KERNEL_OPS = ops
    return ops


_OPS = _register_ops()
Z_HINGE = _OPS["Z_HINGE2_ANT"]
XT_SUM = _OPS["XT_SUM_ANT"]
PX_MASK = _OPS["PX_MASK_ANT"]
U_MASK = _OPS["U_MASK_ANT"]


def _act_set_id(nc):
    from concourse.hw_specs import get_activation_tables

    return list(get_activation_tables(nc.m.arch)).index("natural_log_exp_and_others")


def build_nc(schedule):
    """schedule: tuple of per-block hinge-slot counts (>= 1)."""
    nc = bacc.Bacc("TRN2", target_bir_lowering=False, debug=False)
    xp_dram = nc.dram_tensor("xp", [P, NBLK * CB], F32, kind="ExternalInput")
    out_dram = nc.dram_tensor("out", [1, 1], F32, kind="ExternalOutput")
    xp_ap = xp_dram.ap()

    with tile.TileContext(nc) as tc:
        with (
            tc.tile_pool(name="const", bufs=1) as cpool,
            tc.tile_pool(name="inp", bufs=3) as ipool,
            tc.tile_pool(name="work", bufs=3) as wpool,
            tc.tile_pool(name="zp", bufs=3) as zpool,
            tc.tile_pool(name="tt", bufs=2) as tpool,
            tc.tile_pool(name="accs", bufs=1) as apool,
            tc.tile_pool(name="ps", bufs=1, space="PSUM") as pspool,
        ):
            nc.scalar.add_instruction(
                mybir.InstLoadActFuncSet(
                    name=nc.get_next_instruction_name(), ins=[], outs=[],
                    act_func_set_id=_act_set_id(nc),
                )
            )
            ones = cpool.tile([P, 1], F32, tag="ones")
            nc.vector.memset(ones[:], 1.0)
            hcols = apool.tile([P, NBLK], F32, tag="hcols")
            xtg = apool.tile([P, NGRP], F32, tag="xtg")
            acols = apool.tile([P, NGRP - 1 + GRP], F32, tag="acols")
            lcols = apool.tile([P, NGRP - 1 + GRP], F32, tag="lcols")
            cs_x = pspool.tile([1, 4 * V], F32, tag="cs_x")
            cs_x0 = pspool.tile([1, V], F32, tag="cs_x0")

            for g in range(NGRP):
                tfat = tpool.tile([P, GRP * 16], F32, tag="tfat")
                nc.gpsimd.memset(tfat[:], 0.0)
                px_refs = []
                u_refs = []

                if g == 0:
                    # fast path: per-block DMA/mask/ACT/PE so the engines
                    # start as soon as the first 128 KiB lands
                    for j in range(GRP):
                        blk = j
                        xb = ipool.tile([P, CB], F32, tag="xb")
                        nc.sync.dma_start(
                            xb[:], xp_ap[:, blk * CB : (blk + 1) * CB]
                        )
                        x = xb[:, 0:V]
                        pos = xb[:, V:CB]
                        pxb = wpool.tile([P, V], F32, tag="pxb")
                        nc.vector._custom_dve(
                            PX_MASK, out=pxb[:], in0=x, in1=pos, s0=BIG
                        )
                        ub = wpool.tile([P, V], F32, tag="ub")
                        nc.vector._custom_dve(
                            U_MASK, out=ub[:], in0=x, in1=pos, s0=BIG
                        )
                        px_refs.append(pxb[:])
                        u_refs.append(ub[:])
                        nc.tensor.matmul(
                            cs_x0[:], ones[:], x,
                            start=(j == 0), stop=(j == GRP - 1),
                        )
                        a = wpool.tile([P, V], F32, tag="a0")
                        nc.scalar.activation(
                            a[:], x, ACTF.Abs, bias=0.0, scale=1.0,
                            accum_out=acols[:, GRP - 1 + j : GRP + j],
                        )
                        e = wpool.tile([P, V], F32, tag="e0")
                        nc.scalar.activation(
                            e[:], a[:], ACTF.Exp, bias=0.0, scale=-1.0
                        )
                        lns = wpool.tile([P, V], F32, tag="l0")
                        nc.scalar.activation(
                            lns[:], e[:], ACTF.Ln, bias=1.0, scale=1.0,
                            accum_out=lcols[:, GRP - 1 + j : GRP + j],
                        )
                else:
                    xg = ipool.tile([P, CG], F32, tag="xg")
                    nc.sync.dma_start(xg[:], xp_ap[:, g * CG : (g + 1) * CG])
                    xgv = xg[:].rearrange("p (j c) -> p j c", j=GRP)
                    x_all = xgv[:, :, 0:V]          # [P, GRP, V] strided
                    pos_all = xgv[:, :, V:CB]

                    pxf = wpool.tile([P, GRP * V], F32, tag="pxf")
                    pxv = pxf[:].rearrange("p (j c) -> p j c", j=GRP)
                    nc.vector._custom_dve(
                        PX_MASK, out=pxv, in0=x_all, in1=pos_all, s0=BIG
                    )
                    uf = wpool.tile([P, GRP * V], F32, tag="uf")
                    ufv = uf[:].rearrange("p (j c) -> p j c", j=GRP)
                    nc.vector._custom_dve(
                        U_MASK, out=ufv, in0=x_all, in1=pos_all, s0=BIG
                    )
                    for j in range(GRP):
                        px_refs.append(pxf[:, j * V : (j + 1) * V])
                        u_refs.append(uf[:, j * V : (j + 1) * V])

                    # PE: global column sums of x (strided rhs over the group)
                    nc.tensor.matmul(
                        cs_x[:], ones[:], x_all,
                        start=(g == 1), stop=(g == NGRP - 1),
                    )

                    # ACT chain over the whole group, accums per group
                    af = wpool.tile([P, GRP * V], F32, tag="af")
                    afv = af[:].rearrange("p (j c) -> p j c", j=GRP)
                    nc.scalar.activation(
                        afv, x_all, ACTF.Abs, bias=0.0, scale=1.0,
                        accum_out=acols[:, g - 1 : g],
                    )
                    ef = wpool.tile([P, GRP * V], F32, tag="ef")
                    nc.scalar.activation(ef[:], af[:], ACTF.Exp, bias=0.0, scale=-1.0)
                    lf = wpool.tile([P, GRP * V], F32, tag="lf")
                    nc.scalar.activation(
                        lf[:], ef[:], ACTF.Ln, bias=1.0, scale=1.0,
                        accum_out=lcols[:, g - 1 : g],
                    )

                # extraction per block
                for j in range(GRP):
                    blk = g * GRP + j
                    S = schedule[blk]
                    c0 = j * 16
                    pxb = px_refs[j]
                    rounds = (S + 7) // 8
                    nc.vector.max(tfat[:, c0 : c0 + 8], pxb)
                    src = pxb
                    for r in range(1, rounds):
                        mr = wpool.tile([P, V], F32, tag="mr")
                        nc.vector.match_replace(
                            mr[:], tfat[:, c0 + 8 * (r - 1) : c0 + 8 * r], src, 0.0
                        )
                        nc.vector.max(tfat[:, c0 + 8 * r : c0 + 8 * (r + 1)], mr[:])
                        src = mr[:]

                # sum of positive logits for the group, one op
                xt_scr = tpool.tile([P, GRP * 16], F32, tag="xt_scr")
                nc.vector._custom_dve(
                    XT_SUM, out=xt_scr[:], in0=tfat[:],
                    s0=BIG / 2, s1=BIG,
                    accum_out=xtg[:, g : g + 1],
                )

                # fused hinge per block
                for j in range(GRP):
                    blk = g * GRP + j
                    S = schedule[blk]
                    c0 = j * 16
                    zr = zpool.tile([P, S * V], F32, tag="zr")
                    zv = zr[:].rearrange("p (s n) -> p s n", s=S)
                    u_b = u_refs[j].unsqueeze(1).broadcast_to([P, S, V])
                    t_b = tfat[:, c0 : c0 + S].unsqueeze(2).broadcast_to([P, S, V])
                    nc.vector._custom_dve(
                        Z_HINGE, out=zv, in0=u_b, in1=t_b,
                        s0=BIG / 2, s1=BIG + 1.0,
                        accum_out=hcols[:, blk : blk + 1],
                    )

            # ---- end-of-core combine ----
            h1 = apool.tile([P, 1], F32, tag="h1")
            nc.vector.tensor_reduce(h1[:], hcols[:], AXL.X, ALU.add)
            xt1 = apool.tile([P, 1], F32, tag="xt1")
            nc.vector.tensor_reduce(xt1[:], xtg[:], AXL.X, ALU.add)
            a1 = apool.tile([P, 1], F32, tag="a1")
            nc.vector.tensor_reduce(a1[:], acols[:], AXL.X, ALU.add)
            l1 = apool.tile([P, 1], F32, tag="l1")
            nc.vector.tensor_reduce(l1[:], lcols[:], AXL.X, ALU.add)

            # w = 0.5*a1 + l1 - xt1 + (0.3/0.7)*h1  (per partition)
            w1 = apool.tile([P, 1], F32, tag="w1")
            nc.vector.scalar_tensor_tensor(
                w1[:], a1[:], 0.5, l1[:], ALU.mult, ALU.add
            )
            w2 = apool.tile([P, 1], F32, tag="w2")
            nc.vector.tensor_tensor(w2[:], w1[:], xt1[:], ALU.subtract)
            w3 = apool.tile([P, 1], F32, tag="w3")
            nc.vector.scalar_tensor_tensor(
                w3[:], h1[:], MLM_W / BCE_W, w2[:], ALU.mult, ALU.add
            )
            wps = pspool.tile([1, 1], F32, tag="wps")
            nc.tensor.matmul(wps[:], ones[:], w3[:], start=True, stop=True)
            wsb = apool.tile([1, 1], F32, tag="wsb")
            nc.scalar.copy(wsb[:], wps[:])

            csb = apool.tile([1, 4 * V], F32, tag="csb")
            nc.scalar.copy(csb[:], cs_x[:])
            sxa = apool.tile([1, 1], F32, tag="sxa")
            nc.vector.tensor_reduce(sxa[:], csb[:], AXL.X, ALU.add)
            csb0 = apool.tile([1, V], F32, tag="csb0")
            nc.scalar.copy(csb0[:], cs_x0[:])
            sxb = apool.tile([1, 1], F32, tag="sxb")
            nc.vector.tensor_reduce(sxb[:], csb0[:], AXL.X, ALU.add)
            sx = apool.tile([1, 1], F32, tag="sx")
            nc.vector.tensor_tensor(sx[:], sxa[:], sxb[:], ALU.add)
            t2 = apool.tile([1, 1], F32, tag="t2")
            nc.vector.scalar_tensor_tensor(
                t2[:], sx[:], 0.5, wsb[:], ALU.mult, ALU.add
            )
            o2 = apool.tile([1, 1], F32, tag="o2")
            nc.vector.tensor_scalar(o2[:], t2[:], BCE_W / V, None, ALU.mult)
            nc.sync.dma_start(out_dram.ap()[:, :], o2[:])

    nc.compile()
    return nc


_NC_CACHE = {}


def _get_nc(schedule):
    if schedule not in _NC_CACHE:
        _NC_CACHE[schedule] = build_nc(schedule)
    return _NC_CACHE[schedule]


def _shard(x, t):
    """npos-sorted round-robin shard, x|pos interleave, block-major packing.
    Returns (schedule, [per-core [P, NBLK*CB] arrays])."""
    npos = (t > 0.5).sum(axis=1)
    order = np.argsort(npos, kind="stable")
    npos_sorted = npos[order]
    schedule = tuple(
        max(1, int(npos_sorted[(b + 1) * (N_CORES * P) - 1])) for b in range(NBLK)
    )
    xp = np.concatenate([x, t], axis=1)[order]   # [ROWS, 256]
    shards = []
    for c in range(N_CORES):
        s = xp[c::N_CORES]                        # [RPC, 256] npos-sorted
        s = s.reshape(NBLK, P, CB).transpose(1, 0, 2).reshape(P, NBLK * CB)
        shards.append(np.ascontiguousarray(s))
    return schedule, shards


def kernel(logits: np.ndarray, targets: np.ndarray) -> np.ndarray:
    x = np.asarray(logits, dtype=np.float32).reshape(ROWS, V)
    t = np.asarray(targets, dtype=np.float32).reshape(ROWS, V)
    schedule, shards = _shard(x, t)
    nc = _get_nc(schedule)
    in_maps = [{"xp": shards[c]} for c in range(N_CORES)]
    res = run_bass_kernel_spmd(nc, in_maps, list(range(N_CORES)))
    total = sum(float(res.results[c]["out"][0, 0]) for c in range(N_CORES))
    return np.float32(total / ROWS)



# revision 4
# speedup vs baseline: 1.3212x; 1.3212x over previous
"""Trainium2 Bass kernel for 0.7*BCEWithLogits + 0.3*MultiLabelMarginLoss.

Math (per row of N = B*T rows, V = 128 classes; output = mean over rows):
  bce_row = mean_n[ softplus(x_n) - x_n*t_n ]
  mlm_row = (1/V) sum_{p in pos} sum_{n in neg} relu(1 - x_p + x_n)

Only global sums matter (scalar output), so every term accumulates into
per-block/per-group columns and combines once per core.

Sharding: host sorts rows by positive count, deals them round-robin to the
8 cores (identical npos profile per core -> one NEFF for all cores), and
packs each core's 16 blocks side-by-side as x [128, 16*128]. The sparse
positives (~4/row) are shipped as a gathered table tb [128, 16*16]: slot
(b,k) holds the k-th positive logit of that row-block (verbatim x value),
pads = 1e9. No targets tensor and no on-device extraction needed.

Device math per block (S = max positives in block, from the host schedule):
  main[p] = sum_{k<S} sum_{n in V} relu(x_n - t_k + 1)      (pads: relu(-1e9)=0)
  corr[p] = sum_{k<S} sum_{l in 16} [t_l<100]*relu(t_l - t_k + 1)
  hinge   = main - corr          (n==p pairs cancel exactly)
one fused custom-DVE instruction each; BCE's sum softplus(x) is one ACT
pass per 4-block group (accum_out), sum x*t is one table-wide custom op.
The final combine folds everything into a [1,1] via a ones matmul.

All arithmetic is on device; the host only permutes/gathers/shards the
input values and sums the 8 scalar core partials.
"""

import sys

sys.path.insert(0, "/opt/trn_rl_repo")

import numpy as np

import concourse.bacc as bacc
import concourse.tile as tile
from concourse import mybir
from concourse.bass_utils import run_bass_kernel_spmd

F32 = mybir.dt.float32
ALU = mybir.AluOpType
ACTF = mybir.ActivationFunctionType
AXL = mybir.AxisListType

B, T, V = 16, 1024, 128
ROWS = B * T
N_CORES = 8
RPC = ROWS // N_CORES             # 2048 rows per core
P = 128                           # rows per block
NBLK = RPC // P                   # 16 blocks
GRP = 4                           # blocks per group
NGRP = NBLK // GRP
SLOTS = 16                        # positive-table slots per block

PADV = 1.0e9                      # table pad value (kills both hinge sides)
C0V = 100.0                       # validity threshold (|x| <= ~6 always)
BCE_W = 0.7
MLM_W = 0.3

USE_SOFTPLUS = False              # softplus spline not in the act tables here


def _register_ops():
    from concourse import dve_ops as dops
    from concourse.dve_spec import (
        Spec, Src0, Src1, AluOp, relu, select, Zero, C0, C1,
    )

    if hasattr(dops, "ANT_KERNEL_OPS2"):
        return dops.ANT_KERNEL_OPS2

    def _href(in0, in1, c0, c1, c2):
        a = in0.astype(np.float32).reshape(in0.shape[0], -1)
        b = in1.astype(np.float32).reshape(in0.shape[0], -1)
        z = np.where(a < c0, np.maximum(a - b + c1, 0.0), 0.0)
        return z, z.sum(-1, keepdims=True)

    hinge_spec = Spec(
        body=select(Src0 < C0, relu(Src0 - Src1 + C1), Zero),
        accum=AluOp.ADD, reference=_href,
    )

    def _psref(in0, in1, c0, c1, c2):
        a = in0.astype(np.float32).reshape(in0.shape[0], -1)
        z = np.where(a < c0, a, 0.0)
        return z, z.sum(-1, keepdims=True)

    possum_spec = Spec(
        body=select(Src0 < C0, Src0, Zero),
        accum=AluOp.ADD, reference=_psref,
    )

    ops = {}
    for name, spec in (
        ("HINGE_LT_ANT", hinge_spec),
        ("POSSUM_LT_ANT", possum_spec),
    ):
        opc = max(dops._SUB_OPCODE_FOR_NAME.values()) + 1
        shas = {}
        for ver in ("v3", "v4"):
            r = dops.DveOpSpec(
                name=name, opcode=opc,
                uops=dops.lower(spec, ver=ver), rd1_en=dops.has_src1(spec),
            )
            shas[ver] = r.sha(ver)
        op = dops.DveOp(name, spec, subdim=False, uops_sha=shas)
        dops.OPS.append(op)
        dops.CUSTOM_DVE_SPECS[name] = spec
        dops._SUB_OPCODE_FOR_NAME[name] = opc
        ops[name] = op
    dops.ANT_KERNEL_OPS2 = ops
    return ops


_OPS = _register_ops()
HINGE = _OPS["HINGE_LT_ANT"]
POSSUM = _OPS["POSSUM_LT_ANT"]


def _act_set_id(nc, name):
    from concourse.hw_specs import get_activation_tables

    return list(get_activation_tables(nc.m.arch)).index(name)


def build_nc(schedule):
    """schedule: tuple of per-block hinge-slot counts (>= 1)."""
    nc = bacc.Bacc("TRN2", target_bir_lowering=False, debug=False)
    xg_dram = nc.dram_tensor("xg", [P, NBLK * V], F32, kind="ExternalInput")
    tb_dram = nc.dram_tensor("tb", [P, NBLK * SLOTS], F32, kind="ExternalInput")
    out_dram = nc.dram_tensor("out", [1, 1], F32, kind="ExternalOutput")
    xg_ap = xg_dram.ap()

    with tile.TileContext(nc) as tc:
        with (
            tc.tile_pool(name="const", bufs=1) as cpool,
            tc.tile_pool(name="inp", bufs=3) as ipool,
            tc.tile_pool(name="work", bufs=3) as wpool,
            tc.tile_pool(name="zp", bufs=3) as zpool,
            tc.tile_pool(name="accs", bufs=1) as apool,
            tc.tile_pool(name="ps", bufs=1, space="PSUM") as pspool,
        ):
            set_name = (
                "softplus_and_others" if USE_SOFTPLUS
                else "natural_log_exp_and_others"
            )
            nc.scalar.add_instruction(
                mybir.InstLoadActFuncSet(
                    name=nc.get_next_instruction_name(), ins=[], outs=[],
                    act_func_set_id=_act_set_id(nc, set_name),
                )
            )
            ones = cpool.tile([P, 1], F32, tag="ones")
            nc.vector.memset(ones[:], 1.0)

            tbl = cpool.tile([P, NBLK * SLOTS], F32, tag="tbl")
            nc.sync.dma_start(tbl[:], tb_dram.ap()[:, :])

            hcols = apool.tile([P, NBLK], F32, tag="hcols")
            ccols = apool.tile([P, NBLK], F32, tag="ccols")
            spcols = apool.tile([P, NGRP], F32, tag="spcols")
            xt1 = apool.tile([P, 1], F32, tag="xt1")

            # sum of positive logits over the whole table, one op
            xt_scr = apool.tile([P, NBLK * SLOTS], F32, tag="xt_scr")
            nc.vector._custom_dve(
                POSSUM, out=xt_scr[:], in0=tbl[:], s0=C0V,
                accum_out=xt1[:],
            )

            for g in range(NGRP):
                xg = ipool.tile([P, GRP * V], F32, tag="xg")
                nc.sync.dma_start(
                    xg[:], xg_ap[:, g * GRP * V : (g + 1) * GRP * V]
                )

                # BCE: sum softplus(x) over the group, single ACT pass
                if USE_SOFTPLUS:
                    sp = wpool.tile([P, GRP * V], F32, tag="sp")
                    nc.scalar.activation(
                        sp[:], xg[:], ACTF.Softplus, bias=0.0, scale=1.0,
                        accum_out=spcols[:, g : g + 1],
                    )
                else:
                    e = wpool.tile([P, GRP * V], F32, tag="e")
                    nc.scalar.activation(
                        e[:], xg[:], ACTF.Exp, bias=0.0, scale=1.0
                    )
                    lns = wpool.tile([P, GRP * V], F32, tag="l")
                    nc.scalar.activation(
                        lns[:], e[:], ACTF.Ln, bias=1.0, scale=1.0,
                        accum_out=spcols[:, g : g + 1],
                    )

                for j in range(GRP):
                    blk = g * GRP + j
                    S = schedule[blk]
                    c0 = blk * SLOTS
                    x_blk = xg[:, j * V : (j + 1) * V]
                    t_s = tbl[:, c0 : c0 + S]

                    # main: sum_n relu(x_n - t_k + 1) over all V columns
                    zr = zpool.tile([P, S * V], F32, tag="zr")
                    zv = zr[:].rearrange("p (s n) -> p s n", s=S)
                    nc.vector._custom_dve(
                        HINGE, out=zv,
                        in0=x_blk.unsqueeze(1).broadcast_to([P, S, V]),
                        in1=t_s.unsqueeze(2).broadcast_to([P, S, V]),
                        s0=C0V, s1=1.0,
                        accum_out=hcols[:, blk : blk + 1],
                    )

                    # corr: sum over positive-positive pairs (tiny)
                    cr = wpool.tile([P, S * SLOTS], F32, tag="cr")
                    cv = cr[:].rearrange("p (s l) -> p s l", s=S)
                    nc.vector._custom_dve(
                        HINGE, out=cv,
                        in0=tbl[:, c0 : c0 + SLOTS]
                        .unsqueeze(1).broadcast_to([P, S, SLOTS]),
                        in1=t_s.unsqueeze(2).broadcast_to([P, S, SLOTS]),
                        s0=C0V, s1=1.0,
                        accum_out=ccols[:, blk : blk + 1],
                    )

            # ---- end-of-core combine ----
            h1 = apool.tile([P, 1], F32, tag="h1")
            nc.vector.tensor_reduce(h1[:], hcols[:], AXL.X, ALU.add)
            c1 = apool.tile([P, 1], F32, tag="c1")
            nc.vector.tensor_reduce(c1[:], ccols[:], AXL.X, ALU.add)
            sp1 = apool.tile([P, 1], F32, tag="sp1")
            nc.vector.tensor_reduce(sp1[:], spcols[:], AXL.X, ALU.add)

            d1 = apool.tile([P, 1], F32, tag="d1")
            nc.vector.tensor_tensor(d1[:], sp1[:], xt1[:], ALU.subtract)
            d2 = apool.tile([P, 1], F32, tag="d2")
            nc.vector.tensor_tensor(d2[:], h1[:], c1[:], ALU.subtract)
            # w = (BCE_W/MLM_W)*d1 + d2; final scale MLM_W/V
            w1 = apool.tile([P, 1], F32, tag="w1")
            nc.vector.scalar_tensor_tensor(
                w1[:], d1[:], BCE_W / MLM_W, d2[:], ALU.mult, ALU.add
            )
            wps = pspool.tile([1, 1], F32, tag="wps")
            nc.tensor.matmul(wps[:], ones[:], w1[:], start=True, stop=True)
            wsb = apool.tile([1, 1], F32, tag="wsb")
            nc.scalar.copy(wsb[:], wps[:])
            o2 = apool.tile([1, 1], F32, tag="o2")
            nc.vector.tensor_scalar(o2[:], wsb[:], MLM_W / V, None, ALU.mult)
            nc.sync.dma_start(out_dram.ap()[:, :], o2[:])

    nc.compile()
    return nc


_NC_CACHE = {}


def _get_nc(schedule):
    if schedule not in _NC_CACHE:
        _NC_CACHE[schedule] = build_nc(schedule)
    return _NC_CACHE[schedule]


def _pack_blocks(a):
    """[RPC, W] row-major -> [P, NBLK*W] with blocks side by side."""
    w = a.shape[1]
    return np.ascontiguousarray(
        a.reshape(NBLK, P, w).transpose(1, 0, 2).reshape(P, NBLK * w)
    )


def _shard(x, t):
    """npos-sorted round-robin shard. Returns (schedule, in_maps) where
    in_maps[c] = {"xg": [P, NBLK*V], "tb": [P, NBLK*SLOTS]}."""
    npos = (t > 0.5).sum(axis=1)
    assert npos.max() <= SLOTS, f"row with {npos.max()} positives > {SLOTS}"
    order = np.argsort(npos, kind="stable")
    npos_sorted = npos[order]
    schedule = tuple(
        max(1, int(npos_sorted[(b + 1) * (N_CORES * P) - 1]))
        for b in range(NBLK)
    )
    xs = x[order]
    ps = t[order] > 0.5
    ns = npos_sorted
    in_maps = []
    for c in range(N_CORES):
        xc = xs[c::N_CORES]                       # [RPC, V]
        pc = ps[c::N_CORES]
        nc_ = ns[c::N_CORES]
        # gather positive x values into SLOTS columns (pads = PADV)
        colorder = np.argsort(~pc, axis=1, kind="stable")[:, :SLOTS]
        vals = np.take_along_axis(xc, colorder, axis=1)
        mask = np.arange(SLOTS)[None, :] < nc_[:, None]
        tbl = np.where(mask, vals, np.float32(PADV)).astype(np.float32)
        in_maps.append({"xg": _pack_blocks(xc), "tb": _pack_blocks(tbl)})
    return schedule, in_maps


def kernel(logits: np.ndarray, targets: np.ndarray) -> np.ndarray:
    x = np.asarray(logits, dtype=np.float32).reshape(ROWS, V)
    t = np.asarray(targets, dtype=np.float32).reshape(ROWS, V)
    schedule, in_maps = _shard(x, t)
    nc = _get_nc(schedule)
    res = run_bass_kernel_spmd(nc, in_maps, list(range(N_CORES)))
    total = sum(float(res.results[c]["out"][0, 0]) for c in range(N_CORES))
    return np.float32(total / ROWS)


# revision 5
# speedup vs baseline: 1.5754x; 1.1924x over previous
"""Trainium2 Bass kernel for 0.7*BCEWithLogits + 0.3*MultiLabelMarginLoss.

Math (per row of N = B*T rows, V = 128 classes; output = mean over rows):
  bce_row = mean_n[ softplus(x_n) - x_n*t_n ]
  mlm_row = (1/V) sum_{p in pos} sum_{n in neg} relu(1 - x_p + x_n)

Only global sums matter (scalar output), so every term accumulates into
per-block columns of one [P, 19] tile and combines once per core.

Sharding: host sorts rows by positive count, deals them round-robin to the
8 cores (identical npos profile per core -> one NEFF for all cores), and
packs each core's 16 row-blocks side-by-side as u [128, 16*128], where u
is x with the ~4 positives per row masked to -30 (softplus(-30) ~ 1e-13).
The positive logits are shipped as a gathered table tb [128, 16*16]:
slot (b,k) = k-th positive logit of that row (verbatim), pads = 1e9.

Device math:
  hinge_blk[p] = sum_{k<S} sum_{n in V} relu(u_n - t_k + 1)
    (pads: relu(u-1e9)=0; masked positives: relu(-30-t+1)=0; so this IS
     the pos x neg pairwise sum -- no correction term needed)
  bce_sum = sum_n softplus(u_n) + sum_pos softplus(-x_p)
    (softplus(x)-x = softplus(-x) folds the x*t term away; both sums via
     one exp + one ln(1+e) ACT pass with accum_out, scale=-1 for the
     table so its 1e9 pads underflow to exp->0, ln->0)
one fused custom-DVE instruction per block for the hinge (S slots from
the host-derived schedule); the tail folds everything into [1,1] with a
column scale + ones-matmul + reduce.

All arithmetic is on device; the host only permutes/gathers/shards the
input values (verbatim or constant fills) and sums the 8 core partials.
"""

import sys

sys.path.insert(0, "/opt/trn_rl_repo")

import numpy as np

import concourse.bacc as bacc
import concourse.tile as tile
from concourse import mybir
from concourse.bass_utils import run_bass_kernel_spmd

F32 = mybir.dt.float32
ALU = mybir.AluOpType
ACTF = mybir.ActivationFunctionType
AXL = mybir.AxisListType

B, T, V = 16, 1024, 128
ROWS = B * T
N_CORES = 8
RPC = ROWS // N_CORES             # 2048 rows per core
P = 128                           # rows per block
NBLK = RPC // P                   # 16 blocks
SG = 8                            # blocks per super-group (ACT/DMA chunk)
NSG = NBLK // SG
SLOTS = 16                        # positive-table slots per block

PADV = 1.0e9                      # table pad value (kills hinge, exp(-pad)=0)
NEGV = -30.0                      # masked-positive value in u
BCE_W = 0.7
MLM_W = 0.3


def _register_ops():
    from concourse import dve_ops as dops
    from concourse.dve_spec import Spec, Src0, Src1, AluOp, relu, C1

    if hasattr(dops, "ANT_KERNEL_OPS3"):
        return dops.ANT_KERNEL_OPS3

    def _href(in0, in1, c0, c1, c2):
        a = in0.astype(np.float32).reshape(in0.shape[0], -1)
        b = in1.astype(np.float32).reshape(in0.shape[0], -1)
        z = np.maximum(a - b + c1, 0.0)
        return z, z.sum(-1, keepdims=True)

    hinge_spec = Spec(
        body=relu(Src0 - Src1 + C1),
        accum=AluOp.ADD, reference=_href,
    )

    ops = {}
    for name, spec in (("HINGE_R_ANT", hinge_spec),):
        opc = max(dops._SUB_OPCODE_FOR_NAME.values()) + 1
        shas = {}
        for ver in ("v3", "v4"):
            r = dops.DveOpSpec(
                name=name, opcode=opc,
                uops=dops.lower(spec, ver=ver), rd1_en=dops.has_src1(spec),
            )
            shas[ver] = r.sha(ver)
        op = dops.DveOp(name, spec, subdim=False, uops_sha=shas)
        dops.OPS.append(op)
        dops.CUSTOM_DVE_SPECS[name] = spec
        dops._SUB_OPCODE_FOR_NAME[name] = opc
        ops[name] = op
    dops.ANT_KERNEL_OPS3 = ops
    return ops


_OPS = _register_ops()
HINGE = _OPS["HINGE_R_ANT"]


def _act_set_id(nc, name):
    from concourse.hw_specs import get_activation_tables

    return list(get_activation_tables(nc.m.arch)).index(name)


def build_nc(schedule):
    """schedule: tuple of per-block hinge-slot counts (>= 1)."""
    nc = bacc.Bacc("TRN2", target_bir_lowering=False, debug=False)
    xg_dram = nc.dram_tensor("xg", [P, NBLK * V], F32, kind="ExternalInput")
    tb_dram = nc.dram_tensor("tb", [P, NBLK * SLOTS], F32, kind="ExternalInput")
    out_dram = nc.dram_tensor("out", [1, 1], F32, kind="ExternalOutput")
    xg_ap = xg_dram.ap()

    # acc columns: 0..NBLK-1 scaled hinge, NBLK..NBLK+NSG-1 softplus(u),
    # NBLK+NSG table softplus(-x_p)
    NACC = NBLK + NSG + 1

    with tile.TileContext(nc) as tc:
        with (
            tc.tile_pool(name="const", bufs=1) as cpool,
            tc.tile_pool(name="inp", bufs=2) as ipool,
            tc.tile_pool(name="work", bufs=2) as wpool,
            tc.tile_pool(name="zp", bufs=3) as zpool,
            tc.tile_pool(name="accs", bufs=1) as apool,
            tc.tile_pool(name="ps", bufs=1, space="PSUM") as pspool,
        ):
            nc.scalar.add_instruction(
                mybir.InstLoadActFuncSet(
                    name=nc.get_next_instruction_name(), ins=[], outs=[],
                    act_func_set_id=_act_set_id(
                        nc, "natural_log_exp_and_others"
                    ),
                )
            )
            ones = cpool.tile([P, 1], F32, tag="ones")
            nc.gpsimd.memset(ones[:], 1.0)

            tbl = cpool.tile([P, NBLK * SLOTS], F32, tag="tbl")
            nc.sync.dma_start(tbl[:], tb_dram.ap()[:, :])

            hraw = apool.tile([P, NBLK], F32, tag="hraw")
            acc = apool.tile([P, NACC], F32, tag="acc")

            # BCE positives: sum softplus(-x_p) from the table (pads -> 0)
            te = wpool.tile([P, NBLK * SLOTS], F32, tag="te")
            nc.scalar.activation(te[:], tbl[:], ACTF.Exp, bias=0.0, scale=-1.0)
            tl = wpool.tile([P, NBLK * SLOTS], F32, tag="tl")
            nc.scalar.activation(
                tl[:], te[:], ACTF.Ln, bias=1.0, scale=1.0,
                accum_out=acc[:, NBLK + NSG : NBLK + NSG + 1],
            )

            for g in range(NSG):
                xg = ipool.tile([P, SG * V], F32, tag="xg")
                nc.sync.dma_start(
                    xg[:], xg_ap[:, g * SG * V : (g + 1) * SG * V]
                )

                # BCE negatives: sum softplus(u) over the super-group
                e = wpool.tile([P, SG * V], F32, tag="e")
                nc.scalar.activation(e[:], xg[:], ACTF.Exp, bias=0.0, scale=1.0)
                lns = wpool.tile([P, SG * V], F32, tag="l")
                nc.scalar.activation(
                    lns[:], e[:], ACTF.Ln, bias=1.0, scale=1.0,
                    accum_out=acc[:, NBLK + g : NBLK + g + 1],
                )

                for j in range(SG):
                    blk = g * SG + j
                    S = schedule[blk]
                    x_blk = xg[:, j * V : (j + 1) * V]
                    t_s = tbl[:, blk * SLOTS : blk * SLOTS + S]

                    zr = zpool.tile([P, S * V], F32, tag="zr")
                    zv = zr[:].rearrange("p (s n) -> p s n", s=S)
                    nc.vector._custom_dve(
                        HINGE, out=zv,
                        in0=x_blk.unsqueeze(1).broadcast_to([P, S, V]),
                        in1=t_s.unsqueeze(2).broadcast_to([P, S, V]),
                        s1=1.0,
                        accum_out=hraw[:, blk : blk + 1],
                    )

            # ---- end-of-core combine ----
            # acc[:, 0:NBLK] = hraw * (MLM_W/BCE_W); columns then sum with
            # uniform weight BCE_W/V applied at the very end.
            nc.vector.tensor_scalar(
                acc[:, 0:NBLK], hraw[:], MLM_W / BCE_W, None, ALU.mult
            )
            cps = pspool.tile([1, NACC], F32, tag="cps")
            nc.tensor.matmul(cps[:], ones[:], acc[:], start=True, stop=True)
            s1 = apool.tile([1, 1], F32, tag="s1")
            nc.vector.tensor_reduce(s1[:], cps[:], AXL.X, ALU.add)
            o2 = apool.tile([1, 1], F32, tag="o2")
            nc.vector.tensor_scalar(o2[:], s1[:], BCE_W / V, None, ALU.mult)
            nc.sync.dma_start(out_dram.ap()[:, :], o2[:])

    nc.compile()
    return nc


_NC_CACHE = {}


def _get_nc(schedule):
    if schedule not in _NC_CACHE:
        _NC_CACHE[schedule] = build_nc(schedule)
    return _NC_CACHE[schedule]


def _pack_blocks(a):
    """[RPC, W] row-major -> [P, NBLK*W] with blocks side by side."""
    w = a.shape[1]
    return np.ascontiguousarray(
        a.reshape(NBLK, P, w).transpose(1, 0, 2).reshape(P, NBLK * w)
    )


def _shard(x, t):
    """npos-sorted round-robin shard. Returns (schedule, in_maps) where
    in_maps[c] = {"xg": [P, NBLK*V], "tb": [P, NBLK*SLOTS]}."""
    npos = (t > 0.5).sum(axis=1)
    assert npos.max() <= SLOTS, f"row with {npos.max()} positives > {SLOTS}"
    order = np.argsort(npos, kind="stable")
    npos_sorted = npos[order]
    schedule = tuple(
        max(1, int(npos_sorted[(b + 1) * (N_CORES * P) - 1]))
        for b in range(NBLK)
    )
    xs = x[order]
    ps = t[order] > 0.5
    ns = npos_sorted
    in_maps = []
    for c in range(N_CORES):
        xc = xs[c::N_CORES]                       # [RPC, V]
        pc = ps[c::N_CORES]
        nc_ = ns[c::N_CORES]
        # u: mask positives to NEGV
        uc = np.where(pc, np.float32(NEGV), xc).astype(np.float32)
        # gather positive x values into SLOTS columns (pads = PADV)
        colorder = np.argsort(~pc, axis=1, kind="stable")[:, :SLOTS]
        vals = np.take_along_axis(xc, colorder, axis=1)
        mask = np.arange(SLOTS)[None, :] < nc_[:, None]
        tbl = np.where(mask, vals, np.float32(PADV)).astype(np.float32)
        in_maps.append({"xg": _pack_blocks(uc), "tb": _pack_blocks(tbl)})
    return schedule, in_maps


def kernel(logits: np.ndarray, targets: np.ndarray) -> np.ndarray:
    x = np.asarray(logits, dtype=np.float32).reshape(ROWS, V)
    t = np.asarray(targets, dtype=np.float32).reshape(ROWS, V)
    schedule, in_maps = _shard(x, t)
    nc = _get_nc(schedule)
    res = run_bass_kernel_spmd(nc, in_maps, list(range(N_CORES)))
    total = sum(float(res.results[c]["out"][0, 0]) for c in range(N_CORES))
    return np.float32(total / ROWS)


# revision 6
# speedup vs baseline: 1.7406x; 1.1049x over previous
"""Trainium2 Bass kernel for 0.7*BCEWithLogits + 0.3*MultiLabelMarginLoss.

Math (per row of N = B*T rows, V = 128 classes; output = mean over rows):
  bce_row = mean_n[ softplus(x_n) - x_n*t_n ]
  mlm_row = (1/V) sum_{p in pos} sum_{n in neg} relu(1 - x_p + x_n)

Only global sums matter (scalar output), so every term accumulates into
per-block columns of one [P, 19] tile and combines once per core.

Sharding: host sorts rows by positive count, deals them round-robin to the
8 cores (identical npos profile per core -> one NEFF for all cores), and
packs each core's 16 row-blocks side-by-side as u [128, 16*128], where u
is x with the ~4 positives per row masked to -30 (softplus(-30) ~ 1e-13).
The positive logits are shipped as a gathered table tb [128, 16*16]:
slot (b,k) = k-th positive logit of that row (verbatim), pads = 1e9.

Device math:
  hinge_blk[p] = sum_{k<S} sum_{n in V} relu(u_n - t_k + 1)
    (pads: relu(u-1e9)=0; masked positives: relu(-30-t+1)=0; so this IS
     the pos x neg pairwise sum -- no correction term needed)
  bce_sum = sum_n softplus(u_n) + sum_pos softplus(-x_p)
    (softplus(x)-x = softplus(-x) folds the x*t term away; both sums via
     one exp + one ln(1+e) ACT pass with accum_out, scale=-1 for the
     table so its 1e9 pads underflow to exp->0, ln->0)
one fused custom-DVE instruction per block for the hinge (S slots from
the host-derived schedule); the tail folds everything into [1,1] with a
column scale + ones-matmul + reduce.

All arithmetic is on device; the host only permutes/gathers/shards the
input values (verbatim or constant fills) and sums the 8 core partials.
"""

import sys

sys.path.insert(0, "/opt/trn_rl_repo")

import numpy as np
import ml_dtypes

import concourse.bacc as bacc
import concourse.tile as tile
from concourse import mybir
from concourse.bass_utils import run_bass_kernel_spmd

F32 = mybir.dt.float32
BF16 = mybir.dt.bfloat16
ALU = mybir.AluOpType
ACTF = mybir.ActivationFunctionType
AXL = mybir.AxisListType

B, T, V = 16, 1024, 128
ROWS = B * T
N_CORES = 8
RPC = ROWS // N_CORES             # 2048 rows per core
P = 128                           # rows per block
NBLK = RPC // P                   # 16 blocks
SG = 8                            # blocks per super-group (ACT granularity)
DCH = 4                           # blocks per DMA chunk
NSG = NBLK // SG
SLOTS = 16                        # positive-table slots per block

PADV = 1.0e9                      # table pad value (kills hinge, exp(-pad)=0)
NEGV = -30.0                      # masked-positive value in u
BCE_W = 0.7
MLM_W = 0.3


def _register_ops():
    from concourse import dve_ops as dops
    from concourse.dve_spec import Spec, Src0, Src1, AluOp, relu, C1

    if hasattr(dops, "ANT_KERNEL_OPS3"):
        return dops.ANT_KERNEL_OPS3

    def _href(in0, in1, c0, c1, c2):
        a = in0.astype(np.float32).reshape(in0.shape[0], -1)
        b = in1.astype(np.float32).reshape(in0.shape[0], -1)
        z = np.maximum(a - b + c1, 0.0)
        return z, z.sum(-1, keepdims=True)

    hinge_spec = Spec(
        body=relu(Src0 - Src1 + C1),
        accum=AluOp.ADD, reference=_href,
    )

    ops = {}
    for name, spec in (("HINGE_R_ANT", hinge_spec),):
        opc = max(dops._SUB_OPCODE_FOR_NAME.values()) + 1
        shas = {}
        for ver in ("v3", "v4"):
            r = dops.DveOpSpec(
                name=name, opcode=opc,
                uops=dops.lower(spec, ver=ver), rd1_en=dops.has_src1(spec),
            )
            shas[ver] = r.sha(ver)
        op = dops.DveOp(name, spec, subdim=False, uops_sha=shas)
        dops.OPS.append(op)
        dops.CUSTOM_DVE_SPECS[name] = spec
        dops._SUB_OPCODE_FOR_NAME[name] = opc
        ops[name] = op
    dops.ANT_KERNEL_OPS3 = ops
    return ops


_OPS = _register_ops()
HINGE = _OPS["HINGE_R_ANT"]


def _act_set_id(nc, name):
    from concourse.hw_specs import get_activation_tables

    return list(get_activation_tables(nc.m.arch)).index(name)


def build_nc(schedule):
    """schedule: tuple of per-block hinge-slot counts (>= 1)."""
    nc = bacc.Bacc("TRN2", target_bir_lowering=False, debug=False)
    xg_dram = nc.dram_tensor("xg", [P, NBLK * V], BF16, kind="ExternalInput")
    tb_dram = nc.dram_tensor("tb", [P, NBLK * SLOTS], BF16, kind="ExternalInput")
    out_dram = nc.dram_tensor("out", [1, 1], F32, kind="ExternalOutput")
    xg_ap = xg_dram.ap()

    # acc columns: 0..NBLK-1 scaled hinge, NBLK..NBLK+NSG-1 softplus(u),
    # NBLK+NSG table softplus(-x_p)
    NACC = NBLK + NSG + 1

    with tile.TileContext(nc) as tc:
        with (
            tc.tile_pool(name="const", bufs=1) as cpool,
            tc.tile_pool(name="inp", bufs=2) as ipool,
            tc.tile_pool(name="work", bufs=2) as wpool,
            tc.tile_pool(name="zp", bufs=3) as zpool,
            tc.tile_pool(name="accs", bufs=1) as apool,
            tc.tile_pool(name="ps", bufs=1, space="PSUM") as pspool,
        ):
            nc.scalar.add_instruction(
                mybir.InstLoadActFuncSet(
                    name=nc.get_next_instruction_name(), ins=[], outs=[],
                    act_func_set_id=_act_set_id(
                        nc, "natural_log_exp_and_others"
                    ),
                )
            )
            ones = cpool.tile([P, 1], F32, tag="ones")
            nc.gpsimd.memset(ones[:], 1.0)

            tbl = cpool.tile([P, NBLK * SLOTS], BF16, tag="tbl")
            nc.sync.dma_start(tbl[:], tb_dram.ap()[:, :])

            hraw = apool.tile([P, NBLK], F32, tag="hraw")
            acc = apool.tile([P, NACC], F32, tag="acc")

            # BCE positives: sum softplus(-x_p) from the table (pads -> 0)
            te = wpool.tile([P, NBLK * SLOTS], F32, tag="te")
            nc.scalar.activation(te[:], tbl[:], ACTF.Exp, bias=0.0, scale=-1.0)
            tl = wpool.tile([P, NBLK * SLOTS], F32, tag="tl")
            nc.scalar.activation(
                tl[:], te[:], ACTF.Ln, bias=1.0, scale=1.0,
                accum_out=acc[:, NBLK + NSG : NBLK + NSG + 1],
            )

            xall = cpool.tile([P, NBLK * V], BF16, tag="xall")
            for c in range(NBLK // DCH):
                nc.sync.dma_start(
                    xall[:, c * DCH * V : (c + 1) * DCH * V],
                    xg_ap[:, c * DCH * V : (c + 1) * DCH * V],
                )

            for g in range(NSG):
                xg = xall[:, g * SG * V : (g + 1) * SG * V]

                # BCE negatives: sum softplus(u) over the super-group
                e = wpool.tile([P, SG * V], F32, tag="e")
                nc.scalar.activation(e[:], xg, ACTF.Exp, bias=0.0, scale=1.0)
                lns = wpool.tile([P, SG * V], F32, tag="l")
                nc.scalar.activation(
                    lns[:], e[:], ACTF.Ln, bias=1.0, scale=1.0,
                    accum_out=acc[:, NBLK + g : NBLK + g + 1],
                )

                for j in range(SG):
                    blk = g * SG + j
                    S = schedule[blk]
                    x_blk = xg[:, j * V : (j + 1) * V]
                    t_s = tbl[:, blk * SLOTS : blk * SLOTS + S]

                    zr = zpool.tile([P, S * V], F32, tag="zr")
                    zv = zr[:].rearrange("p (s n) -> p s n", s=S)
                    nc.vector._custom_dve(
                        HINGE, out=zv,
                        in0=x_blk.unsqueeze(1).broadcast_to([P, S, V]),
                        in1=t_s.unsqueeze(2).broadcast_to([P, S, V]),
                        s1=1.0,
                        accum_out=hraw[:, blk : blk + 1],
                    )

            # ---- end-of-core combine ----
            # acc[:, 0:NBLK] = hraw * (MLM_W/BCE_W); columns then sum with
            # uniform weight BCE_W/V applied at the very end.
            nc.vector.tensor_scalar(
                acc[:, 0:NBLK], hraw[:], MLM_W / BCE_W, None, ALU.mult
            )
            cps = pspool.tile([1, NACC], F32, tag="cps")
            nc.tensor.matmul(cps[:], ones[:], acc[:], start=True, stop=True)
            s1 = apool.tile([1, 1], F32, tag="s1")
            nc.vector.tensor_reduce(s1[:], cps[:], AXL.X, ALU.add)
            nc.sync.dma_start(out_dram.ap()[:, :], s1[:])

    nc.compile()
    return nc


_NC_CACHE = {}


def _get_nc(schedule):
    if schedule not in _NC_CACHE:
        _NC_CACHE[schedule] = build_nc(schedule)
    return _NC_CACHE[schedule]


def _pack_blocks(a):
    """[RPC, W] row-major -> [P, NBLK*W] with blocks side by side."""
    w = a.shape[1]
    return np.ascontiguousarray(
        a.reshape(NBLK, P, w).transpose(1, 0, 2).reshape(P, NBLK * w)
    )


def _shard(x, t):
    """npos-sorted round-robin shard. Returns (schedule, in_maps) where
    in_maps[c] = {"xg": [P, NBLK*V], "tb": [P, NBLK*SLOTS]}."""
    npos = (t > 0.5).sum(axis=1)
    assert npos.max() <= SLOTS, f"row with {npos.max()} positives > {SLOTS}"
    order = np.argsort(npos, kind="stable")
    npos_sorted = npos[order]
    schedule = tuple(
        max(1, int(npos_sorted[(b + 1) * (N_CORES * P) - 1]))
        for b in range(NBLK)
    )
    xs = x[order]
    ps = t[order] > 0.5
    ns = npos_sorted
    in_maps = []
    for c in range(N_CORES):
        xc = xs[c::N_CORES]                       # [RPC, V]
        pc = ps[c::N_CORES]
        nc_ = ns[c::N_CORES]
        # u: mask positives to NEGV
        uc = np.where(pc, np.float32(NEGV), xc).astype(ml_dtypes.bfloat16)
        # gather positive x values into SLOTS columns (pads = PADV)
        colorder = np.argsort(~pc, axis=1, kind="stable")[:, :SLOTS]
        vals = np.take_along_axis(xc, colorder, axis=1)
        mask = np.arange(SLOTS)[None, :] < nc_[:, None]
        tbl = np.where(mask, vals, np.float32(PADV)).astype(ml_dtypes.bfloat16)
        in_maps.append({"xg": _pack_blocks(uc), "tb": _pack_blocks(tbl)})
    return schedule, in_maps


def kernel(logits: np.ndarray, targets: np.ndarray) -> np.ndarray:
    x = np.asarray(logits, dtype=np.float32).reshape(ROWS, V)
    t = np.asarray(targets, dtype=np.float32).reshape(ROWS, V)
    schedule, in_maps = _shard(x, t)
    nc = _get_nc(schedule)
    res = run_bass_kernel_spmd(nc, in_maps, list(range(N_CORES)))
    total = sum(float(res.results[c]["out"][0, 0]) for c in range(N_CORES))
    return np.float32(total * (BCE_W / V) / ROWS)


# revision 7
# speedup vs baseline: 1.7641x; 1.0135x over previous
"""Trainium2 Bass kernel for 0.7*BCEWithLogits + 0.3*MultiLabelMarginLoss.

Math (per row of N = B*T rows, V = 128 classes; output = mean over rows):
  bce_row = mean_n[ softplus(x_n) - x_n*t_n ]
  mlm_row = (1/V) sum_{p in pos} sum_{n in neg} relu(1 - x_p + x_n)

Only global sums matter (scalar output), so every term accumulates into
per-block columns of one [P, 19] tile and combines once per core.

Sharding: host sorts rows by positive count, deals them round-robin to the
8 cores (identical npos profile per core -> one NEFF for all cores), and
packs each core's 16 row-blocks side-by-side as u [128, 16*128], where u
is x with the ~4 positives per row masked to -30 (softplus(-30) ~ 1e-13).
The positive logits are shipped as a gathered table tb [128, 16*16]:
slot (b,k) = k-th positive logit of that row (verbatim), pads = 1e9.

Device math:
  hinge_blk[p] = sum_{k<S} sum_{n in V} relu(u_n - t_k + 1)
    (pads: relu(u-1e9)=0; masked positives: relu(-30-t+1)=0; so this IS
     the pos x neg pairwise sum -- no correction term needed)
  bce_sum = sum_n softplus(u_n) + sum_pos softplus(-x_p)
    (softplus(x)-x = softplus(-x) folds the x*t term away; both sums via
     one exp + one ln(1+e) ACT pass with accum_out, scale=-1 for the
     table so its 1e9 pads underflow to exp->0, ln->0)
one fused custom-DVE instruction per block for the hinge (S slots from
the host-derived schedule); the tail folds everything into [1,1] with a
column scale + ones-matmul + reduce.

All arithmetic is on device; the host only permutes/gathers/shards the
input values (verbatim or constant fills) and sums the 8 core partials.
"""

import sys

sys.path.insert(0, "/opt/trn_rl_repo")

import numpy as np
import ml_dtypes

import concourse.bacc as bacc
import concourse.tile as tile
from concourse import mybir
from concourse.bass_utils import run_bass_kernel_spmd

F32 = mybir.dt.float32
BF16 = mybir.dt.bfloat16
ALU = mybir.AluOpType
ACTF = mybir.ActivationFunctionType
AXL = mybir.AxisListType

B, T, V = 16, 1024, 128
ROWS = B * T
N_CORES = 8
RPC = ROWS // N_CORES             # 2048 rows per core
P = 128                           # rows per block
NBLK = RPC // P                   # 16 blocks
SG = 8                            # blocks per super-group (ACT granularity)
DCH = 4                           # blocks per DMA chunk
NSG = NBLK // SG
SLOTS = 16                        # positive-table slots per block

PADV = 1.0e9                      # table pad value (kills hinge, exp(-pad)=0)
NEGV = -30.0                      # masked-positive value in u
BCE_W = 0.7
MLM_W = 0.3


def _register_ops():
    from concourse import dve_ops as dops
    from concourse.dve_spec import Spec, Src0, Src1, AluOp, relu, C1

    if hasattr(dops, "ANT_KERNEL_OPS3"):
        return dops.ANT_KERNEL_OPS3

    def _href(in0, in1, c0, c1, c2):
        a = in0.astype(np.float32).reshape(in0.shape[0], -1)
        b = in1.astype(np.float32).reshape(in0.shape[0], -1)
        z = np.maximum(a - b + c1, 0.0)
        return z, z.sum(-1, keepdims=True)

    hinge_spec = Spec(
        body=relu(Src0 - Src1 + C1),
        accum=AluOp.ADD, reference=_href,
    )

    ops = {}
    for name, spec in (("HINGE_R_ANT", hinge_spec),):
        opc = max(dops._SUB_OPCODE_FOR_NAME.values()) + 1
        shas = {}
        for ver in ("v3", "v4"):
            r = dops.DveOpSpec(
                name=name, opcode=opc,
                uops=dops.lower(spec, ver=ver), rd1_en=dops.has_src1(spec),
            )
            shas[ver] = r.sha(ver)
        op = dops.DveOp(name, spec, subdim=False, uops_sha=shas)
        dops.OPS.append(op)
        dops.CUSTOM_DVE_SPECS[name] = spec
        dops._SUB_OPCODE_FOR_NAME[name] = opc
        ops[name] = op
    dops.ANT_KERNEL_OPS3 = ops
    return ops


_OPS = _register_ops()
HINGE = _OPS["HINGE_R_ANT"]


def _act_set_id(nc, name):
    from concourse.hw_specs import get_activation_tables

    return list(get_activation_tables(nc.m.arch)).index(name)


def build_nc(schedule):
    """schedule: tuple of per-block hinge-slot counts (>= 1)."""
    nc = bacc.Bacc("TRN2", target_bir_lowering=False, debug=False)
    # single input: [table (NBLK*SLOTS) | u (NBLK*V)] so chunk 0 lands both
    TCOLS = NBLK * SLOTS
    XCOLS = TCOLS + NBLK * V
    xg_dram = nc.dram_tensor("xg", [P, XCOLS], BF16, kind="ExternalInput")
    out_dram = nc.dram_tensor("out", [1, 1], F32, kind="ExternalOutput")
    xg_ap = xg_dram.ap()

    # acc columns: 0..NBLK-1 scaled hinge, NBLK..NBLK+NSG-1 softplus(u),
    # NBLK+NSG table softplus(-x_p)
    NACC = NBLK + NSG + 1

    with tile.TileContext(nc) as tc:
        with (
            tc.tile_pool(name="const", bufs=1) as cpool,
            tc.tile_pool(name="inp", bufs=2) as ipool,
            tc.tile_pool(name="work", bufs=2) as wpool,
            tc.tile_pool(name="zp", bufs=3) as zpool,
            tc.tile_pool(name="accs", bufs=1) as apool,
            tc.tile_pool(name="ps", bufs=1, space="PSUM") as pspool,
        ):
            nc.scalar.add_instruction(
                mybir.InstLoadActFuncSet(
                    name=nc.get_next_instruction_name(), ins=[], outs=[],
                    act_func_set_id=_act_set_id(
                        nc, "natural_log_exp_and_others"
                    ),
                )
            )
            ones = cpool.tile([P, 1], F32, tag="ones")
            nc.gpsimd.memset(ones[:], 1.0)

            xall = cpool.tile([P, XCOLS], BF16, tag="xall")
            # chunk 0: table + first DCH blocks; then DCH-block chunks
            cuts = [0, TCOLS + DCH * V]
            while cuts[-1] < XCOLS:
                cuts.append(min(XCOLS, cuts[-1] + DCH * V))
            for a, b in zip(cuts[:-1], cuts[1:]):
                nc.sync.dma_start(xall[:, a:b], xg_ap[:, a:b])
            tbl = xall[:, 0:TCOLS]

            hraw = apool.tile([P, NBLK], F32, tag="hraw")
            acc = apool.tile([P, NACC], F32, tag="acc")

            # BCE positives: sum softplus(-x_p) from the table (pads -> 0)
            te = wpool.tile([P, NBLK * SLOTS], F32, tag="te")
            nc.scalar.activation(te[:], tbl, ACTF.Exp, bias=0.0, scale=-1.0)
            tl = wpool.tile([P, NBLK * SLOTS], F32, tag="tl")
            nc.scalar.activation(
                tl[:], te[:], ACTF.Ln, bias=1.0, scale=1.0,
                accum_out=acc[:, NBLK + NSG : NBLK + NSG + 1],
            )

            for g in range(NSG):
                xg = xall[:, TCOLS + g * SG * V : TCOLS + (g + 1) * SG * V]

                # BCE negatives: sum softplus(u) over the super-group
                e = wpool.tile([P, SG * V], F32, tag="e")
                nc.scalar.activation(e[:], xg, ACTF.Exp, bias=0.0, scale=1.0)
                lns = wpool.tile([P, SG * V], F32, tag="l")
                nc.scalar.activation(
                    lns[:], e[:], ACTF.Ln, bias=1.0, scale=1.0,
                    accum_out=acc[:, NBLK + g : NBLK + g + 1],
                )

                for j in range(SG):
                    blk = g * SG + j
                    S = schedule[blk]
                    x_blk = xg[:, j * V : (j + 1) * V]
                    t_s = xall[:, blk * SLOTS : blk * SLOTS + S]

                    zr = zpool.tile([P, S * V], F32, tag="zr")
                    zv = zr[:].rearrange("p (s n) -> p s n", s=S)
                    nc.vector._custom_dve(
                        HINGE, out=zv,
                        in0=x_blk.unsqueeze(1).broadcast_to([P, S, V]),
                        in1=t_s.unsqueeze(2).broadcast_to([P, S, V]),
                        s1=1.0,
                        accum_out=hraw[:, blk : blk + 1],
                    )

            # ---- end-of-core combine ----
            # acc[:, 0:NBLK] = hraw * (MLM_W/BCE_W); columns then sum with
            # uniform weight BCE_W/V applied at the very end.
            nc.vector.tensor_scalar(
                acc[:, 0:NBLK], hraw[:], MLM_W / BCE_W, None, ALU.mult
            )
            cps = pspool.tile([1, NACC], F32, tag="cps")
            nc.tensor.matmul(cps[:], ones[:], acc[:], start=True, stop=True)
            s1 = apool.tile([1, 1], F32, tag="s1")
            nc.vector.tensor_reduce(s1[:], cps[:], AXL.X, ALU.add)
            nc.sync.dma_start(out_dram.ap()[:, :], s1[:])

    nc.compile()
    return nc


_NC_CACHE = {}


def _get_nc(schedule):
    if schedule not in _NC_CACHE:
        _NC_CACHE[schedule] = build_nc(schedule)
    return _NC_CACHE[schedule]


def _pack_blocks(a):
    """[RPC, W] row-major -> [P, NBLK*W] with blocks side by side."""
    w = a.shape[1]
    return np.ascontiguousarray(
        a.reshape(NBLK, P, w).transpose(1, 0, 2).reshape(P, NBLK * w)
    )


def _shard(x, t):
    """npos-sorted round-robin shard. Returns (schedule, in_maps) where
    in_maps[c] = {"xg": [P, NBLK*V], "tb": [P, NBLK*SLOTS]}."""
    npos = (t > 0.5).sum(axis=1)
    assert npos.max() <= SLOTS, f"row with {npos.max()} positives > {SLOTS}"
    order = np.argsort(npos, kind="stable")
    npos_sorted = npos[order]
    schedule = tuple(
        max(1, int(npos_sorted[(b + 1) * (N_CORES * P) - 1]))
        for b in range(NBLK)
    )
    xs = x[order]
    ps = t[order] > 0.5
    ns = npos_sorted
    in_maps = []
    for c in range(N_CORES):
        xc = xs[c::N_CORES]                       # [RPC, V]
        pc = ps[c::N_CORES]
        nc_ = ns[c::N_CORES]
        # u: mask positives to NEGV
        uc = np.where(pc, np.float32(NEGV), xc).astype(ml_dtypes.bfloat16)
        # gather positive x values into SLOTS columns (pads = PADV)
        colorder = np.argsort(~pc, axis=1, kind="stable")[:, :SLOTS]
        vals = np.take_along_axis(xc, colorder, axis=1)
        mask = np.arange(SLOTS)[None, :] < nc_[:, None]
        tbl = np.where(mask, vals, np.float32(PADV)).astype(ml_dtypes.bfloat16)
        in_maps.append(
            {"xg": np.concatenate([_pack_blocks(tbl), _pack_blocks(uc)], axis=1)}
        )
    return schedule, in_maps


def kernel(logits: np.ndarray, targets: np.ndarray) -> np.ndarray:
    x = np.asarray(logits, dtype=np.float32).reshape(ROWS, V)
    t = np.asarray(targets, dtype=np.float32).reshape(ROWS, V)
    schedule, in_maps = _shard(x, t)
    nc = _get_nc(schedule)
    res = run_bass_kernel_spmd(nc, in_maps, list(range(N_CORES)))
    total = sum(float(res.results[c]["out"][0, 0]) for c in range(N_CORES))
    return np.float32(total * (BCE_W / V) / ROWS)
